# revision 12
# baseline (speedup 1.0000x reference)
"""Trainium2 Bass kernel for the Conv2.5d depth-masked convolution problem.

Math (per batch b, output pixel (y,x), f scalar):
  d0 = depth[b,0,y,x]; s0 = d0/f
  For tap (i,j) in 3x3 window, dw = depth[b,0,y+i-1,x+j-1] (zero-padded):
    level l in {0,1,2} active iff  z_l - s0/2 <= dw < z_l + s0/2,
    z_l = d0 + (l-1)*s0.  The intervals are disjoint and adjacent, so per
    (tap, pixel) at most one level is active.
  out[b,o,y,x] = sum_{l,i,j,c} W[l,o,c,i,j] * inputs[b,c,y+i-1,x+j-1] * mask
                 + bias[o]

Kernel strategy (8 NeuronCores, data-parallel over (batch, y-half)):
  - The HOST replicates the reference's fp32 mask arithmetic bit-exactly
    and encodes, per (tap, pixel), the code m = 0 (no level) or CODES[l]
    (level l active).  CODES are exact sign/power-of-two fp16 values.
  - Cubic Lagrange through T(0)=0, T(CODES[l])=W_l has no constant
    term, so the masked weight selection becomes
      T(m) = p1*m + p2*m^2 + p3*m^3.
    The device builds the power basis per tap-pair with three fp16
    tensor_tensor ops (x1 = m*S, x2 = m*x1, x3 = m*x2) that hit the DVE
    2x fast mode, and accumulates three matmul groups (p1,p2,p3) per
    pair into PSUM.  13 matmul groups total (4 pairs x 3 + center).
  - Tap pairs are stacked across the 128 SBUF partitions (2 taps x 64
    channels).  Mask ops cover 32 output rows at a time (two 16-row PSUM
    chunks), the three column-pairs run on the Vector engine and the row
    pair's chain runs on the GpSimd engine, so the PE consumes 8 matmuls
    per semaphore and both mask engines work in parallel.
  - The center tap is level 1 whenever d0>0; if the dataset contains
    d0==0 pixels, a program variant masks the center rhs by a shipped
    per-pixel indicator.
  - All host->device tiles are packed so every DMA line is one fat
    contiguous descriptor; loads/stores are split into strips across
    DMA queues.
  - fp16 everywhere on the masked path (exact mask codes; image/weight
    quantization ~5e-4 relative), fp32 PSUM accumulate, ScalarE evicts
    with fused bias add.
"""

import numpy as np

import concourse.mybir as mybir
from concourse import bacc
from concourse.tile import TileContext
from concourse.bass_utils import run_bass_kernel_spmd

# ---- problem constants (hardcoded per contest rules) ----
B, CIN, COUT, H, W = 4, 64, 64, 128, 128
KK = 3
N_CORES = 8
HY = H // 2              # rows per core (y-half)

# taps in pair order: pairs (0,1) (2,3) (4,5) stack along columns (delta
# (0,2), served by the col-shifted tile), pair (6,7) stacks along rows
# (delta (2,0), served by the row-shifted tile).
TAPS8 = [(0, 0), (0, 2), (1, 0), (1, 2), (2, 0), (2, 2), (0, 1), (2, 1)]
TROWS, TCOLS = 67, 130   # stacked slab tile: 67 rows x 130 cols per half
TFREE = TROWS * TCOLS    # 8710
SCH = 32                 # output rows per mask supertile (2 psum chunks)
S_PX = SCH * W           # 4096 pixels per supertile
CHY = 16                 # output rows per psum chunk
CH_PX = CHY * W          # 2048 pixels per chunk
SLY = 4                  # psum slice = 4 rows x 128 cols = 512 fp32 (1 bank)
NSL = CHY // SLY         # 4 matmul slices per chunk
NGRP = 13                # center + 4 pairs x 3 power-basis groups
# level codes: no-level -> 0, level l -> CODES[l].  Chosen from exact
# sign/power-of-two fp16 values so the x-chain multiplies are exact, and
# to keep the Lagrange recombination well-conditioned.
CODES = (1.0, -1.0, 2.0)

_CACHE = {}
TRACE = False            # set by test harness to collect an NTFF profile
LAST_EXEC_NS = None
LAST_PROFILE = None


def _build_program(center_masked):
    nc = bacc.Bacc("TRN2", target_bir_lowering=False)
    f16, f32 = mybir.dt.float16, mybir.dt.float32
    i2d = nc.declare_dram_parameter("img2", [128, TFREE], f16, isOutput=False)
    i132d = nc.declare_dram_parameter("img132", [128, TFREE], f16, isOutput=False)
    mcd = nc.declare_dram_parameter("mc", [9, HY * W], f16, isOutput=False)
    wpd = nc.declare_dram_parameter("wp", [128, NGRP * 64], f16, isOutput=False)
    biad = nc.declare_dram_parameter("bia", [COUT, 1], f32, isOutput=False)
    outd = nc.declare_dram_parameter("out", [COUT, HY, W], f32, isOutput=True)

    mult = mybir.AluOpType.mult

    with TileContext(nc) as tc:
        with tc.tile_pool(name="w", bufs=1) as wpool, \
             tc.tile_pool(name="slab", bufs=1) as spool, \
             tc.tile_pool(name="m", bufs=1) as mpool, \
             tc.tile_pool(name="xv", bufs=6) as xvpool, \
             tc.tile_pool(name="xp", bufs=3) as xppool, \
             tc.tile_pool(name="ow", bufs=2) as opool, \
             tc.tile_pool(name="psum", bufs=1, space="PSUM") as pspool:

            wt = wpool.tile([128, NGRP * 64], f16)
            nc.sync.dma_start(out=wt[:], in_=wpd[:, :])
            bt = wpool.tile([COUT, 1], f32)
            nc.sync.dma_start(out=bt[:], in_=biad[:, :])
            i2 = spool.tile([128, TFREE], f16, tag="i2")
            i132 = spool.tile([128, TFREE], f16, tag="i132")
            # strip the big loads across DMA queues
            for k in range(4):
                a, b = k * 2180, min((k + 1) * 2180, TFREE)
                nc.sync.dma_start(out=i2[:, a:b], in_=i2d[:, a:b])
                nc.sync.dma_start(out=i132[:, a:b], in_=i132d[:, a:b])
            i2v = i2.rearrange("p (r c) -> p r c", r=TROWS)
            i132v = i132.rearrange("p (r c) -> p r c", r=TROWS)

            def lhsT(g, k128=True):
                v = wt[:, g * 64:(g + 1) * 64]
                return v if k128 else wt[0:64, g * 64:(g + 1) * 64]

            for sp in range(HY // SCH):
                ry = sp * SCH
                o0 = ry * W
                mts = []
                for p in range(4):
                    mt = mpool.tile([128, S_PX], f16, tag=f"m{p}")
                    nc.sync.dma_start(
                        out=mt[0:64, :],
                        in_=mcd[2 * p:2 * p + 1, o0:o0 + S_PX].to_broadcast([64, S_PX]))
                    nc.sync.dma_start(
                        out=mt[64:128, :],
                        in_=mcd[2 * p + 1:2 * p + 2, o0:o0 + S_PX].to_broadcast([64, S_PX]))
                    mts.append(mt.rearrange("p (y x) -> p y x", y=SCH))
                if center_masked:
                    mtc = mpool.tile([64, S_PX], f16, tag="mc")
                    nc.sync.dma_start(
                        out=mtc[:, :],
                        in_=mcd[8:9, o0:o0 + S_PX].to_broadcast([64, S_PX]))

                # power-basis tiles: pairs 0-2 on Vector, pair 3 on GpSimd
                xs = [[None] * 3 for _ in range(4)]
                prev = [i2v[:, ry + p: ry + p + SCH, 0:W] for p in range(3)]
                prev.append(i132v[:, ry: ry + SCH, 1: 1 + W])
                for j in range(3):
                    for p in range(3):
                        x = xvpool.tile([128, S_PX], f16, tag="x")
                        xv = x.rearrange("p (y x) -> p y x", y=SCH)
                        nc.vector.tensor_tensor(out=xv, in0=mts[p], in1=prev[p], op=mult)
                        xs[p][j] = xv
                        prev[p] = xv
                    x = xppool.tile([128, S_PX], f16, tag="xp")
                    xv = x.rearrange("p (y x) -> p y x", y=SCH)
                    nc.gpsimd.tensor_tensor(out=xv, in0=mts[3], in1=prev[3], op=mult)
                    xs[3][j] = xv
                    prev[3] = xv

                if center_masked:
                    xc = xvpool.tile([64, S_PX], f16, tag="xc")
                    xcv = xc.rearrange("p (y x) -> p y x", y=SCH)
                    nc.vector.tensor_tensor(
                        out=xcv, in0=mtc.rearrange("p (y x) -> p y x", y=SCH),
                        in1=i132v[0:64, ry + 1: ry + 1 + SCH, 1: 1 + W], op=mult)

                pss = [pspool.tile([COUT, CH_PX], mybir.dt.float32,
                                   name=f"ps{c}", tag=f"ps{c}")
                       for c in range(2)]
                psvs = [ps.rearrange("p (y x) -> p y x", y=CHY) for ps in pss]

                def mm(lh, rhs3, g, c):
                    # rhs3 is a [*, SCH, W] view; chunk c covers rows
                    # c*CHY .. (c+1)*CHY
                    for s in range(NSL):
                        r0 = c * CHY + s * SLY
                        nc.tensor.matmul(
                            psvs[c][:, s * SLY:(s + 1) * SLY, :], lh,
                            rhs3[:, r0:r0 + SLY, :],
                            start=(g == 0), stop=(g == NGRP - 1))

                # group order: center first (no mask dependency), then the
                # power-basis groups in production order; both chunks of a
                # group run back-to-back off one semaphore set.
                for c in range(2):
                    if center_masked:
                        mm(lhsT(0, False), xcv, 0, c)
                    else:
                        cw = i132v[0:64, ry + 1: ry + 1 + SCH, 1: 1 + W]
                        mm(lhsT(0, False), cw, 0, c)
                for j in range(3):
                    for p in range(4):
                        g = 1 + j * 4 + p
                        for c in range(2):
                            mm(lhsT(g), xs[p][j], g, c)

                for c in range(2):
                    ot = opool.tile([COUT, CH_PX], f32, tag="o")
                    nc.scalar.activation(
                        out=ot[:], in_=pss[c][:],
                        func=mybir.ActivationFunctionType.Identity, bias=bt[:])
                    ry2 = ry + c * CHY
                    otv = ot[:].rearrange("p (y x) -> p y x", y=CHY)
                    nc.scalar.dma_start(
                        out=outd[:, ry2:ry2 + 8, :], in_=otv[:, 0:8, :])
                    nc.scalar.dma_start(
                        out=outd[:, ry2 + 8:ry2 + CHY, :], in_=otv[:, 8:CHY, :])

    nc.finalize()
    return nc


def _pack_weights(weight):
    """Lagrange power-basis weights: group 0 is the center W1 [64, 64];
    group 1 + j*4 + p holds p_{j+1} at pair p's two taps stacked."""
    Wl = [np.asarray(weight[l], np.float64) for l in range(KK)]  # [O,C,3,3]
    # solve Vandermonde for p(x) = p1 x + p2 x^2 + p3 x^3 through
    # p(CODES[l]) = W_l (p(0)=0 automatically)
    V = np.array([[u, u * u, u ** 3] for u in CODES], np.float64)
    T = np.stack(Wl, 0).reshape(3, -1)               # [3, O*C*9]
    P = np.linalg.solve(V, T).reshape(3, COUT, CIN, KK, KK)
    Wp = np.zeros((NGRP, 128, 64), np.float64)
    Wp[0, 0:64, :] = Wl[1][:, :, 1, 1].T
    for j in range(3):
        for p in range(4):
            g = 1 + j * 4 + p
            tA, tB = TAPS8[2 * p], TAPS8[2 * p + 1]
            Wp[g, 0:64, :] = P[j][:, :, tA[0], tA[1]].T
            Wp[g, 64:128, :] = P[j][:, :, tB[0], tB[1]].T
    return np.ascontiguousarray(
        Wp.transpose(1, 0, 2).reshape(128, NGRP * 64)).astype(np.float16)


def _host_levels(depth, f):
    """Replicate the reference's fp32 mask chain exactly; return the
    per-tap level codes ([B, 9, H, W] fp16; row 8 is the center
    indicator) plus whether any center pixel has no active level."""
    f32 = np.float32
    d0 = np.asarray(depth, f32)[:, 0]                 # [B,H,W]
    s0 = (d0 / f32(f)).astype(f32)
    half = (s0 / f32(2.0)).astype(f32)
    dpad = np.zeros((B, H + 2, W + 2), f32)
    dpad[:, 1:H + 1, 1:W + 1] = d0
    mc = np.zeros((B, 9, H, W), np.float16)
    ab = []
    for l in range(KK):
        off = f32(l - (KK - 1) / 2.0)
        z0 = (d0 + (off * s0).astype(f32)).astype(f32)
        ab.append(((z0 - half).astype(f32), (z0 + half).astype(f32)))
    for t, (i, j) in enumerate(TAPS8):
        dw = dpad[:, i:i + H, j:j + W]
        code = np.zeros((B, H, W), np.float16)
        for l in range(KK):
            a, b = ab[l]
            code[(dw >= a) & (dw < b)] = np.float16(CODES[l])
        mc[:, t] = code
    # center tap: dw = d0
    a, b = ab[1]
    mc[:, 8][(d0 >= a) & (d0 < b)] = np.float16(1.0)
    center_masked = bool((mc[:, 8] != 1.0).any())
    return mc, center_masked


def _host_pack(inputs, mc):
    """Per-core tiles: img2/img132 ([128, TFREE] fp16, pair-stacked slabs
    with one contiguous DMA line per partition) and mc ([9, HY*W] fp16)."""
    in_maps = []
    for b in range(B):
        Ipad = np.zeros((CIN, H + 6, W + 6), np.float16)
        Ipad[:, 2:H + 2, 2:W + 2] = inputs[b]
        for half in range(2):
            y0 = half * HY
            # slab[r, c] = padded img at (y0-1+r, c-1) = Ipad[:, y0+1+r, 1+c]
            slab = Ipad[:, y0 + 1: y0 + 1 + 69, 1: 1 + 132]      # [C, 69, 132]
            img2 = np.empty((128, TROWS, TCOLS), np.float16)
            img2[0:64] = slab[:, 0:TROWS, 0:TCOLS]
            img2[64:128] = slab[:, 0:TROWS, 2:2 + TCOLS]
            img132 = np.empty((128, TROWS, TCOLS), np.float16)
            img132[0:64] = slab[:, 0:TROWS, 0:TCOLS]
            img132[64:128] = slab[:, 2:2 + TROWS, 0:TCOLS]
            in_maps.append({
                "img2": np.ascontiguousarray(img2.reshape(128, TFREE)),
                "img132": np.ascontiguousarray(img132.reshape(128, TFREE)),
                "mc": np.ascontiguousarray(
                    mc[b, :, y0:y0 + HY, :].reshape(9, HY * W)),
            })
    return in_maps


def kernel(inputs, depth, weight, bias, f):
    inputs = np.ascontiguousarray(np.asarray(inputs, np.float32))
    depth = np.ascontiguousarray(np.asarray(depth, np.float32))
    weight = np.asarray(weight, np.float32)
    bias_np = np.asarray(bias, np.float32).reshape(COUT, 1)
    fv = float(np.asarray(f).item() if hasattr(f, "item") or isinstance(f, np.ndarray) else f)

    mc, center_masked = _host_levels(depth, fv)
    key = ("v4", center_masked)
    if key not in _CACHE:
        _CACHE[key] = _build_program(center_masked)
    nc = _CACHE[key]

    in_maps = _host_pack(inputs, mc)
    Wp = _pack_weights(weight)
    for m in in_maps:
        m["wp"] = Wp
        m["bia"] = bias_np

    global LAST_EXEC_NS, LAST_PROFILE
    res = run_bass_kernel_spmd(nc, in_maps, list(range(N_CORES)), trace=TRACE)
    if TRACE:
        LAST_EXEC_NS = res.exec_time_ns
        LAST_PROFILE = res.profile_json
    outs = [res.results[c]["out"] for c in range(N_CORES)]
    full = np.empty((B, COUT, H, W), np.float32)
    for b in range(B):
        full[b, :, 0:HY, :] = outs[2 * b]
        full[b, :, HY:H, :] = outs[2 * b + 1]
    return full
